# revision 20
# baseline (speedup 1.0000x reference)
"""Trainium2 Bass kernel for EnhancedCondConv2d (moe_routing).

Data-parallel over batch: 8 cores x 2 samples each. Full inputs in,
full outputs back.

v5 design (per core; routing batched across both samples):
  host:     x pre-cast to padded fp8 (conv input), x pre-transposed to
            [w, h, i] bf16 (residual input), exact f32 channel pool
            sums + 3x3 window sums, experts laid out tap-major with
            contiguous i so LDWEIGHTS hits fast-weight-load.
  routing:  both samples' MLP+softmax batched at t=0; the mid-chain
            sigmoid runs as 1/(1+exp(-v)) so the whole routing chain
            uses one ACT table set (preloaded by a dummy exp at t=0);
            wgen for both samples in one pass of 144 FD=16 matmuls,
            started as soon as the first expert tap-pair lands.
  conv(b):  3x3 grouped conv, tap-outer over 8-row rounds (stationary
            weights reused across the round): 4 DoubleRow pairs + 1
            plain fp8 matmul per 4-row bank; ACT eviction (x cw from
            the analytic SE path) to bf16 osb.
  post(b):  per-pixel channel stats via PE transpose matmuls against
            [I|1]; o-columns evicted contiguously into osbT[w, h, o],
            sum column into spsum[w, h]; DVE max-reduce (2x mode) into
            spmax[w, h]; 7x7 spatial conv as 14 banded-Toeplitz
            matmuls; sigmoid -> swT[w, h]; final
            out_T = osbT * swT (free-dim broadcast) + xT (gpsimd add),
            stored transposed; host untransposes.
"""

import math
from contextlib import ExitStack

import numpy as np

import concourse.bass as bass
import concourse.bacc as bacc
import concourse.mybir as mybir
import concourse.tile as tile
from concourse.bass_utils import run_bass_kernel_spmd

F32 = mybir.dt.float32
BF16 = mybir.dt.bfloat16
FP8 = mybir.dt.float8e4
AX = mybir.AxisListType
ALU = mybir.AluOpType
ACTF = mybir.ActivationFunctionType
DR = mybir.MatmulPerfMode.DoubleRow

B, CI, CO, H, W, E, KK, RR = 16, 128, 128, 128, 128, 16, 3, 8
NCORES = 8
BL = B // NCORES  # 2 samples per core
EPS = 1e-5
HW = H * W
BNS = 1.0 / math.sqrt(1.0 + EPS)
HP, WP = H + 2, W + 2  # host-padded
HT = H + 6  # h dim of spsum/spmax with +-3 padding for the 7x7 conv

_CACHE = {}


def _build_module():
    nc = bacc.Bacc("TRN2", target_bir_lowering=False, debug=False)

    x8_d = nc.dram_tensor("x8", [BL, CI, HP, WP], FP8, kind="ExternalInput").ap()
    xt_d = nc.dram_tensor("xt", [BL, W, H, CI], BF16, kind="ExternalInput").ap()
    ew_d = nc.dram_tensor("ew", [128, KK * KK, 16, CI], FP8, kind="ExternalInput").ap()
    mc_d = nc.dram_tensor("mc", [128, 14, 128], BF16, kind="ExternalInput").ap()
    # packed constant blocks (single descriptor-efficient DMA each):
    # f32: pv(2) gs1 bb1 gs2n bb2n gsca1 bbca1 gsca2 bbca2 gssa bssa |
    #      rb3r(16, parts 0:2) rw1t(16) rw3t(16) caw1t(16) |
    #      rw2t(128, parts 0:16) caw2t(128, parts 0:16)
    cf_d = nc.dram_tensor("cf", [128, 332], F32, kind="ExternalInput").ap()
    # bf16: idc(129) | e16t(128, parts 0:16)
    cb_d = nc.dram_tensor("cb", [128, 257], BF16, kind="ExternalInput").ap()
    # fp8: s8(18) | bmask2(16)
    c8_d = nc.dram_tensor("c8", [128, 34], FP8, kind="ExternalInput").ap()

    out_d = nc.dram_tensor("out", [BL, W, H, CO], BF16, kind="ExternalOutput").ap()

    with tile.TileContext(nc) as tc, ExitStack() as ctx:
        _kernel_body(ctx, tc, x8_d, xt_d, ew_d, mc_d, cf_d, cb_d, c8_d, out_d)
    nc.compile()
    return nc


def _kernel_body(ctx, tc, x8_d, xt_d, ew_d, mc_d, cf_d, cb_d, c8_d, out_d):
    nc = tc.nc

    cpool = ctx.enter_context(tc.tile_pool(name="const", bufs=1))
    xpool = ctx.enter_context(tc.tile_pool(name="xp", bufs=2))
    opool = ctx.enter_context(tc.tile_pool(name="op", bufs=1))
    tpool = ctx.enter_context(tc.tile_pool(name="tp", bufs=2))
    wpool = ctx.enter_context(tc.tile_pool(name="wp", bufs=1))
    spool = ctx.enter_context(tc.tile_pool(name="sp", bufs=2))
    fpool = ctx.enter_context(tc.tile_pool(name="fp", bufs=4))

    pconv = ctx.enter_context(tc.tile_pool(name="pc", bufs=3, space="PSUM"))
    pbig = ctx.enter_context(tc.tile_pool(name="pb", bufs=2, space="PSUM"))
    psml = ctx.enter_context(tc.tile_pool(name="ps", bufs=1, space="PSUM"))

    # warm the exp ACT table set at t=0 (overlaps const DMA)
    wrm = spool.tile([1, 1], F32, tag="wrm", bufs=1)
    nc.vector.memset(wrm, 0.0)
    nc.scalar.activation(wrm, wrm, ACTF.Exp)

    # ---------- big loads all on the sync queue (scalar stays pure
    # compute: a DMA issue costs ~650ns of the issuing engine) ----------
    XCH = [(0, 33), (33, 65), (65, 97), (97, 130)]
    cf = cpool.tile([128, 332], F32, tag="cf")
    nc.sync.dma_start(cf, cf_d)
    x8s = []
    for b in range(BL):
        x8 = xpool.tile([128, HP, WP], FP8, tag="x8", name=f"x8_{b}")
        x8s.append(x8)
    for u, (r0, r1) in enumerate(XCH):
        nc.sync.dma_start(x8s[0][:, r0:r1, :], x8_d[0, :, r0:r1, :])
    # experts in tap-pair chunks so wgen starts early
    ecr = cpool.tile([128, KK * KK, 16, CI], FP8, tag="ecr")
    for kt in range(5):
        k1 = min(2 * kt + 2, 9)
        nc.sync.dma_start(ecr[:, 2 * kt:k1], ew_d[:, 2 * kt:k1])
    for u, (r0, r1) in enumerate(XCH):
        nc.sync.dma_start(x8s[1][:, r0:r1, :], x8_d[1, :, r0:r1, :])

    # ---------- packed constants (cf loaded first on sync above) ----------
    cb = cpool.tile([128, 257], BF16, tag="cb")
    nc.gpsimd.dma_start(cb, cb_d)
    c8 = cpool.tile([128, 34], FP8, tag="c8")
    nc.gpsimd.dma_start(c8, c8_d)
    mc = cpool.tile([128, 14, 128], BF16, tag="mc")
    nc.gpsimd.dma_start(mc, mc_d)

    pv = cf[:, 0:2]
    gs1, bb1 = cf[0:16, 2:3], cf[0:16, 3:4]
    gs2n, bb2n = cf[:, 4:5], cf[:, 5:6]
    gsca1, bbca1 = cf[0:16, 6:7], cf[0:16, 7:8]
    gsca2, bbca2 = cf[:, 8:9], cf[:, 9:10]
    gssa, bssa = cf[:, 10:11], cf[:, 11:12]
    rb3r = cf[0:BL, 12:28]
    rw1t = cf[:, 28:44]
    rw3t = cf[:, 44:60]
    caw1t = cf[:, 60:76]
    rw2t = cf[0:16, 76:204]
    caw2t = cf[0:16, 204:332]
    idc = cb[:, 0:129]
    e16t = cb[0:16, 129:257]
    s8 = c8[:, 0:18].rearrange("p (b k) -> p b k", b=BL)
    bmask = c8[:, 18:34]

    # ---------- routing (both samples batched; exp table set only) ----
    def routing():
        mm1 = psml.tile([16, BL], F32, tag="r", bufs=1)
        nc.tensor.matmul(mm1, rw1t, pv, start=True, stop=True)
        h1 = spool.tile([16, BL], F32, tag="h1")
        nc.scalar.activation(h1, mm1, ACTF.Relu, bias=bb1, scale=gs1)
        mm2 = psml.tile([128, BL], F32, tag="r", bufs=1)
        nc.tensor.matmul(mm2, rw2t, h1, start=True, stop=True)
        # sigmoid(v) = 1/(1+exp(-v)) to stay in the exp table set
        eg = spool.tile([128, BL], F32, tag="eg")
        nc.scalar.activation(eg, mm2, ACTF.Exp, bias=bb2n, scale=gs2n)
        gp = spool.tile([128, BL], F32, tag="gp")
        nc.scalar.activation(gp, eg, ACTF.Copy, bias=1.0)
        gg = spool.tile([128, BL], F32, tag="gg")
        nc.vector.reciprocal(gg, gp)
        mm3 = psml.tile([BL, E], F32, tag="r", bufs=1)
        nc.tensor.matmul(mm3, gg, rw3t, start=True, stop=True)
        lg = spool.tile([BL, E], F32, tag="lg")
        nc.vector.tensor_add(lg, mm3, rb3r)
        # logits are O(0.3) here, so plain exp is safe (no max-subtract)
        e16 = spool.tile([BL, E], F32, tag="e16")
        nc.scalar.activation(e16, lg, ACTF.Exp)
        s1 = spool.tile([BL, 1], F32, tag="s1")
        nc.vector.tensor_reduce(s1, e16, AX.X, ALU.add)
        rinv = spool.tile([BL, 1], F32, tag="rinv")
        nc.vector.reciprocal(rinv, s1)
        rwrow = spool.tile([BL, E], BF16, tag="rwrow")
        nc.vector.tensor_scalar_mul(rwrow, e16, rinv)
        # transpose [BL,16] -> [16,BL] on the PE, widen to 16 (j,b) cols
        rwtp = psml.tile([16, BL], BF16, tag="r", bufs=1)
        nc.tensor.matmul(rwtp, rwrow, idc[0:BL, 0:BL], is_transpose=True)
        rwt16 = spool.tile([16, 8, BL], BF16, tag="rwt16", bufs=1)
        nc.vector.tensor_copy(rwt16, rwtp.unsqueeze(1).broadcast_to([16, 8, BL]))
        rwbp = psml.tile([128, 16], F32, tag="r", bufs=1)
        nc.tensor.matmul(rwbp, e16t, rwt16, start=True, stop=True)
        rwblk = spool.tile([128, 16], FP8, tag="rwblk", bufs=1)
        nc.vector.tensor_tensor(rwblk, bmask, rwbp, ALU.mult)
        return rwblk

    # ---------- wgen (both samples): w[i, k, o] stored as w*32, fp8 ----
    def wgen(rwblk):
        wsbs = [wpool.tile([128, 9, CO], FP8, tag=f"wsb{b}", name=f"wsb{b}")
                for b in range(BL)]
        for kt in range(5):  # taps (2kt, 2kt+1); kt=4 -> tap 8 only
            ntap = 1 if kt == 4 else 2
            pwt = pbig.tile([128, 2, 512], F32, tag="big", name=f"pw{kt}")
            for j in range(ntap):
                k = 2 * kt + j
                for og in range(16):
                    nc.tensor.matmul(pwt[:, j, og * 16:og * 16 + 16],
                                     ecr[:, k, og, :], rwblk,
                                     start=True, stop=True)
                for b in range(BL):
                    src = bass.AP(pwt.tensor, pwt.offset + j * 512 + b,
                                  [list(pwt.ap[0]), [16, 16], [2, 8]])
                    nc.vector.tensor_scalar_mul(wsbs[b][:, k, :], src, 2.0)
        return wsbs

    # ---------- SE chain for sample b (analytic channel sums) ----------
    def se_chain(b, wsb):
        pcp = psml.tile([128, 1], F32, tag="r", bufs=1, name=f"pcp{b}")
        for k in range(9):
            nc.tensor.matmul(pcp, wsb[:, k, :], s8[:, b, k].unsqueeze(1),
                             start=(k == 0), stop=(k == 8))
        cpsb = spool.tile([128, 1], F32, tag="cpsb")
        nc.vector.tensor_copy(cpsb, pcp)
        se1 = psml.tile([16, 1], F32, tag="r", bufs=1, name=f"se1_{b}")
        nc.tensor.matmul(se1, caw1t, cpsb, start=True, stop=True)
        ch = spool.tile([16, 1], F32, tag="ch")
        nc.scalar.activation(ch, se1, ACTF.Relu, bias=bbca1, scale=gsca1)
        se2 = psml.tile([128, 1], F32, tag="r", bufs=1, name=f"se2_{b}")
        nc.tensor.matmul(se2, caw2t, ch, start=True, stop=True)
        cw = spool.tile([128, 1], F32, tag="cw")
        nc.scalar.activation(cw, se2, ACTF.Sigmoid, bias=bbca2, scale=gsca2)
        cws = spool.tile([128, 1], F32, tag="cws", name=f"cws{b}")
        nc.vector.tensor_scalar_mul(cws, cw, 1.0 / 32.0)
        return cws

    # conv tap pairs for DoubleRow (kw parity must match; rhs delta even)
    PAIRS = [(0, 3), (1, 4), (2, 5), (6, 8)]
    CH6 = [(6 * c, 6) for c in range(21)] + [(126, 2)]

    def conv_post(b, wsb, cws):
        """Conv rounds with the transpose/stats chunks interleaved, then
        the max tree + 7x7 spatial conv -> swT2."""
        x8 = x8s[b]
        osb = opool.tile([128, H, W], BF16, tag="osb", name=f"osb{b}")
        osbT = tpool.tile([128, H, CO], BF16, tag="osbT", name=f"osbT{b}")
        spsum = tpool.tile([128, HT], BF16, tag="spsum", name=f"spsum{b}")
        spmax = tpool.tile([128, HT], BF16, tag="spmax", name=f"spmax{b}")

        nc.vector.memset(spsum[:, 0:3], 0.0)
        nc.vector.memset(spsum[:, H + 3:H + 6], 0.0)
        nc.vector.memset(spmax[:, 0:3], 0.0)
        nc.vector.memset(spmax[:, H + 3:H + 6], 0.0)

        def emit_round(rnd):
            pcs = [pconv.tile([128, 512], F32, tag="c", name=f"pc{b}_{rnd}_{g}")
                   for g in range(2)]
            for t, (k0, k1) in enumerate(PAIRS):
                kh0, kw0 = k0 // 3, k0 % 3
                kh1, kw1 = k1 // 3, k1 % 3
                delta = (kh1 - kh0) * WP + (kw1 - kw0)
                lhs = bass.AP(wsb.tensor, wsb.offset + k0 * CO,
                              [list(wsb.ap[0]), [(k1 - k0) * CO, 2], [1, CO]])
                for g in range(2):
                    r0 = rnd * 8 + g * 4
                    eloff = (r0 + kh0) * WP + kw0
                    rhs = bass.AP(x8.tensor, x8.offset + eloff,
                                  [list(x8.ap[0]), [delta, 2], [WP, 4], [1, W]])
                    nc.tensor.matmul(pcs[g], lhs, rhs, start=(t == 0),
                                     stop=False, perf_mode=DR)
            # tap 7 (kh=2, kw=1) plain fp8 matmul (FWL path)
            for g in range(2):
                r0 = rnd * 8 + g * 4
                rhs7 = bass.AP(x8.tensor, x8.offset + (r0 + 2) * WP + 1,
                               [list(x8.ap[0]), [WP, 4], [1, W]])
                nc.tensor.matmul(pcs[g], wsb[:, 7, :], rhs7,
                                 start=False, stop=True)
            for g in range(2):
                r0 = rnd * 8 + g * 4
                nc.scalar.activation(
                    osb[:, r0:r0 + 4, :],
                    pcs[g].rearrange("p (a b) -> p a b", a=4),
                    ACTF.Copy, scale=cws)

        def emit_chunk(h0, nr):
            ptt = pbig.tile([128, 2, 512], F32, tag="big", name=f"ptt{b}_{h0}")
            for j in range(nr):
                nc.tensor.matmul(ptt[:, j // 3, (j % 3) * 129:(j % 3) * 129 + 129],
                                 osb[:, h0 + j, :], idc, start=True, stop=True)
            nb = (nr + 2) // 3
            nr3 = min(nr, 3)
            # o-columns -> osbT rows (contiguous), one ACT instr
            src = bass.AP(ptt.tensor, ptt.offset,
                          [list(ptt.ap[0]), [512, nb], [129, nr3], [1, CO]])
            dst = bass.AP(osbT.tensor, osbT.offset + h0 * CO,
                          [list(osbT.ap[0]), [3 * CO, nb], [CO, nr3], [1, CO]])
            nc.scalar.activation(dst, src, ACTF.Copy)
            # sum column -> spsum (tiny, on DVE)
            ssrc = bass.AP(ptt.tensor, ptt.offset + CO,
                           [list(ptt.ap[0]), [512, nb], [129, nr3]])
            sdst = bass.AP(spsum.tensor, spsum.offset + 3 + h0,
                           [list(spsum.ap[0]), [3, nb], [1, nr3]])
            nc.vector.tensor_copy(sdst, ssrc)
            # channel max: pairwise TT tree (TT max has a 2x uop;
            # reduce does not), fully reduced per chunk
            scr = tpool.tile([128, 6, 64], BF16, tag="scr", bufs=2,
                             name=f"scr{b}_{h0}")
            nc.vector.tensor_tensor(scr[:, 0:nr, :],
                                    osbT[:, h0:h0 + nr, 0:64],
                                    osbT[:, h0:h0 + nr, 64:128], ALU.max)
            wd = 32
            while wd >= 1:
                dst = (spmax[:, 3 + h0:3 + h0 + nr] if wd == 1
                       else scr[:, 0:nr, 0:wd])
                nc.vector.tensor_tensor(dst, scr[:, 0:nr, 0:wd],
                                        scr[:, 0:nr, wd:2 * wd], ALU.max)
                wd //= 2

        ci = 0
        for rnd in range(16):
            emit_round(rnd)
            while ci < len(CH6) and CH6[ci][0] + CH6[ci][1] <= 8 * rnd:
                emit_chunk(*CH6[ci])
                ci += 1
        while ci < len(CH6):
            emit_chunk(*CH6[ci])
            ci += 1

        # 7x7 spatial conv: 14 banded Toeplitz matmuls
        psw = pbig.tile([128, 2, 512], F32, tag="big", name=f"psw{b}")
        for t in range(14):
            c, dh = t // 7, t % 7
            src = spsum if c == 0 else spmax
            nc.tensor.matmul(psw[:, 0, 0:128], mc[:, t, :], src[:, dh:dh + 128],
                             start=(t == 0), stop=(t == 13))
        swT = spool.tile([128, 128], BF16, tag="swT", name=f"swT{b}")
        nc.scalar.activation(swT, psw[:, 0, 0:128], ACTF.Sigmoid,
                             bias=bssa, scale=gssa)
        swTf = spool.tile([128, 128], F32, tag="swTf", name=f"swTf{b}")
        nc.vector.tensor_copy(swTf, swT)
        # duplicated-pair copy so the final multiply can run in 2x mode:
        # swT2[w, h, 0:2] = swT[w, h]
        swT2 = spool.tile([128, 128, 2], BF16, tag="swT2", name=f"swT2{b}")
        nc.vector.tensor_copy(swT2, swT.unsqueeze(2).broadcast_to([128, 128, 2]))
        return osbT, swTf, swT2

    def final(b, osbT, swTf, swT2, nact=0):
        # the last `nact` 8-row tiles' multiplies run per-row on the (idle
        # in the tail) scalar engine via the per-partition f32 scale
        foas = []
        for g in range(16 - nact, 16):
            fo = fpool.tile([128, 8, CO], BF16, tag="foa", bufs=4,
                            name=f"foa{b}_{g}")
            for j in range(8):
                h = 8 * g + j
                nc.scalar.activation(fo[:, j, :], osbT[:, h, :], ACTF.Copy,
                                     scale=swTf[:, h:h + 1])
            foas.append(fo)
        nq = (16 - nact) * 8 // 32
        for q in range(nq):  # 32-row quarters
            h0 = 32 * q
            xtt = fpool.tile([128, 32, CI], BF16, tag="xtt", bufs=2,
                             name=f"xt{b}_{q}")
            nc.sync.dma_start(xtt, xt_d[b, :, h0:h0 + 32, :])
            fo = fpool.tile([128, 32, CO], BF16, tag="fo", bufs=2,
                            name=f"fo{b}_{q}")
            # operand2 reads each (w,h) sw value as duplicated bf16
            # pairs (innermost step 1) so 2x_1P packing applies
            op2 = bass.AP(swT2.tensor, swT2.offset + h0 * 2,
                          [list(swT2.ap[0]), [2, 32], [0, 64], [1, 2]])
            op1 = bass.AP(osbT.tensor, osbT.offset + h0 * CO,
                          [list(osbT.ap[0]), [CO, 32], [2, 64], [1, 2]])
            fov = bass.AP(fo.tensor, fo.offset,
                          [list(fo.ap[0]), [CO, 32], [2, 64], [1, 2]])
            nc.vector.tensor_tensor(fov, op1, op2, ALU.mult)
            nc.vector.tensor_tensor(fo, fo, xtt, ALU.add)
            nc.sync.dma_start(out_d[b, :, h0:h0 + 32, :], fo)
        for i, g in enumerate(range(16 - nact, 16)):
            h0 = 8 * g
            xtt = fpool.tile([128, 8, CI], BF16, tag="xta", bufs=2,
                             name=f"xta{b}_{g}")
            nc.sync.dma_start(xtt, xt_d[b, :, h0:h0 + 8, :])
            fo = foas[i]
            nc.vector.tensor_tensor(fo, fo, xtt, ALU.add)
            nc.gpsimd.dma_start(out_d[b, :, h0:h0 + 8, :], fo)

    # ---------- schedule ----------
    rwblk = routing()
    wsbs = wgen(rwblk)
    cws0 = se_chain(0, wsbs[0])
    cws1 = se_chain(1, wsbs[1])
    osbT0, swT0, swT20 = conv_post(0, wsbs[0], cws0)
    osbT1, swT1, swT21 = conv_post(1, wsbs[1], cws1)
    final(0, osbT0, swT0, swT20, nact=0)
    final(1, osbT1, swT1, swT21, nact=4)


def _host_prep(inp):
    import ml_dtypes
    x = np.asarray(inp["x"], np.float32)

    # padded fp8 conv input
    x8 = np.zeros((B, CI, HP, WP), dtype=ml_dtypes.float8_e4m3fn)
    x8[:, :, 1:H + 1, 1:W + 1] = x.astype(ml_dtypes.float8_e4m3fn)
    # transposed bf16 residual input [b, w, h, i]
    xt = np.ascontiguousarray(x.transpose(0, 3, 2, 1)).astype(ml_dtypes.bfloat16)

    # exact channel pool sums [i, b]
    xs = x.sum(axis=(2, 3))  # [B, I]
    # 3x3 window sums of the padded fp8 x (what the conv actually sees)
    x8f = np.zeros((B, CI, HP, WP), np.float32)
    x8f[:, :, 1:H + 1, 1:W + 1] = x8[:, :, 1:H + 1, 1:W + 1].astype(np.float32)
    r1 = x8f[:, :, 1, :].sum(-1)
    r128 = x8f[:, :, H, :].sum(-1)
    c1 = x8f[:, :, :, 1].sum(-1)
    c128 = x8f[:, :, :, W].sum(-1)
    tot = x8f.sum((2, 3))
    s9 = np.zeros((B, CI, 3, 3), np.float32)  # [b, i, kh, kw]
    for kh in range(3):
        a = tot.copy()
        if kh == 0:
            a -= r128
        if kh == 2:
            a -= r1
        for kw in range(3):
            v = a.copy()
            if kw == 0:
                d = c128.copy()
                if kh == 0:
                    d -= x8f[:, :, H, W]
                if kh == 2:
                    d -= x8f[:, :, 1, W]
                v -= d
            if kw == 2:
                d = c1.copy()
                if kh == 0:
                    d -= x8f[:, :, H, 1]
                if kh == 2:
                    d -= x8f[:, :, 1, 1]
                v -= d
            s9[:, :, kh, kw] = v
    s9 = s9 * 0.25

    experts = np.ascontiguousarray(inp["experts"], dtype=np.float32)
    # [E, O, I, K, K] -> [(o_sub 8, e 16)=128, k=9, og=16, i=128]
    ew = experts.reshape(E, 16, 8, CI, 9).transpose(2, 0, 4, 1, 3)
    ew = np.ascontiguousarray(ew).reshape(128, 9, 16, CI)

    idc = np.zeros((128, 129), dtype=np.float32)
    idc[np.arange(128), np.arange(128)] = 1.0
    idc[:, 128] = 1.0

    # banded Toeplitz matrices M[t=(c,dh)][w', w] = tap[c,dh,dw] at
    # w == w' + 3 - dw  (mean channel c=0 scaled by 1/CO)
    saw = np.asarray(inp["sa_w"], np.float32).reshape(2, 7, 7)
    mcm = np.zeros((14, 128, 128), dtype=np.float32)
    for t in range(14):
        c, dh = t // 7, t % 7
        for dw in range(7):
            val = float(saw[c, dh, dw]) * (1.0 / CO if c == 0 else 1.0)
            wp = np.arange(128)
            w = wp + 3 - dw
            m = (w >= 0) & (w < 128)
            mcm[t, wp[m], w[m]] += val
    mc = np.ascontiguousarray(mcm.transpose(1, 0, 2)).astype(ml_dtypes.bfloat16)

    e16t = np.zeros((16, 8, 16), dtype=np.float32)
    for e in range(16):
        e16t[e, :, e] = 1.0
    e16t = e16t.reshape(16, 128)

    # bmask2[p=(osub 8, e 16), col=(j*2+b)] = 1 iff osub == j
    bm = np.zeros((8, 16, 8, BL), dtype=np.float32)
    for j in range(8):
        bm[j, :, j, :] = 1.0
    bm = bm.reshape(128, 16)

    # ---- packed constant blocks ----
    cf = np.zeros((128, 332), np.float32)
    cf[0:16, 2] = np.asarray(inp["rbn1_g"], np.float32) * (BNS / HW)
    cf[0:16, 3] = np.asarray(inp["rbn1_b"], np.float32)
    cf[:, 4] = np.asarray(inp["rbn2_g"], np.float32) * (-BNS)
    cf[:, 5] = -np.asarray(inp["rbn2_b"], np.float32)
    cf[0:16, 6] = np.asarray(inp["ca_bn1_g"], np.float32) * (BNS / HW / 8.0)
    cf[0:16, 7] = np.asarray(inp["ca_bn1_b"], np.float32)
    cf[:, 8] = np.asarray(inp["ca_bn2_g"], np.float32) * BNS
    cf[:, 9] = np.asarray(inp["ca_bn2_b"], np.float32)
    cf[:, 10] = float(np.asarray(inp["sa_bn_g"], np.float32)[0]) * BNS
    cf[:, 11] = float(np.asarray(inp["sa_bn_b"], np.float32)[0])
    cf[0:BL, 12:28] = np.asarray(inp["rb3"], np.float32)[None, :]
    cf[:, 28:44] = np.asarray(inp["rw1"], np.float32).T
    cf[:, 44:60] = np.asarray(inp["rw3"], np.float32).T
    cf[:, 60:76] = np.asarray(inp["ca_w1"], np.float32).T
    cf[0:16, 76:204] = np.asarray(inp["rw2"], np.float32).T
    cf[0:16, 204:332] = np.asarray(inp["ca_w2"], np.float32).T

    cb = np.zeros((128, 257), np.float32)
    cb[:, 0:129] = idc
    cb[0:16, 129:257] = e16t
    cb = cb.astype(ml_dtypes.bfloat16)

    shared = {
        "ew": (ew * 16.0).astype(ml_dtypes.float8_e4m3fn),
        "mc": mc,
        "cb": cb,
    }
    in_maps = []
    for c in range(NCORES):
        m = dict(shared)
        sl = slice(BL * c, BL * (c + 1))
        m["x8"] = np.ascontiguousarray(x8[sl])
        m["xt"] = np.ascontiguousarray(xt[sl])
        cfc = cf.copy()
        cfc[:, 0:2] = xs[sl].T  # pv [i, b]
        m["cf"] = cfc
        c8 = np.zeros((128, 34), np.float32)
        # s8 layout [i, b, 9] with col k = kh*3+kw
        c8[:, 0:18] = s9[sl].transpose(1, 0, 2, 3).reshape(CI, BL * 9)
        c8[:, 18:34] = bm
        m["c8"] = c8.astype(ml_dtypes.float8_e4m3fn)
        in_maps.append(m)
    return in_maps


def _assemble(results):
    out = np.concatenate([r["out"] for r in results], axis=0)  # [B, W, H, O]
    return np.ascontiguousarray(out.transpose(0, 3, 2, 1)).astype(np.float32)


def get_module():
    if "nc" not in _CACHE:
        _CACHE["nc"] = _build_module()
    return _CACHE["nc"]


def kernel(**inputs):
    nc = get_module()
    in_maps = _host_prep(inputs)
    res = run_bass_kernel_spmd(nc, in_maps, core_ids=list(range(NCORES)))
    return _assemble(res.results)


# revision 21
# speedup vs baseline: 1.0084x; 1.0084x over previous
"""Trainium2 Bass kernel for EnhancedCondConv2d (moe_routing).

Data-parallel over batch: 8 cores x 2 samples each. Full inputs in,
full outputs back.

Design (per core; routing batched across both samples):
  host:     x pre-cast to padded fp8 (conv input), x pre-transposed to
            [w, h, i] bf16 (residual input), exact f32 channel pool
            sums + 3x3 window sums, experts laid out tap-major with
            contiguous i (fast-weight-load), all small consts packed
            into three descriptor-efficient blocks (one DMA each).
  routing:  both samples' MLP+softmax batched at t=0; the mid-chain
            sigmoid runs as 1/(1+exp(-v)) so the whole routing chain
            uses one ACT table set (preloaded by a dummy exp at t=0);
            wgen for both samples in one pass of 144 FD=16 matmuls,
            started as soon as the first expert tap-pair lands.
  conv(b):  3x3 grouped conv over 8-row rounds: 4 DoubleRow fp8 pairs
            + 1 plain fp8 matmul per 4-row bank (fp8 DR streaming
            limit); ACT eviction (x cw from the analytic SE path) to
            bf16 osb. Transpose/stats chunks are interleaved between
            rounds so the PE never idles and sample b's attention map
            is ready right after its conv.
  post(b):  per-pixel channel stats via PE transpose matmuls against
            [I|1]; o-columns evicted contiguously into osbT[w, h, o],
            sum column into spsum[w, h]; channel max via per-chunk
            pairwise TT-max tree (TT has a 2x uop, reduce does not);
            7x7 spatial conv as 14 banded-Toeplitz matmuls ->
            sigmoid -> swT[w, h].
  final(b): out_T = osbT * swT + xT in 32-row tiles on DVE, with the
            multiply reading sw as duplicated bf16 pairs (2x mode, no
            partition broadcast anywhere); for the last sample part of
            the multiplies run per-row on the then-idle scalar engine;
            stored transposed, host untransposes.

All DMAs are issued from sync/gpsimd so the scalar engine stays pure
compute (a DMA issue costs ~650ns of the issuing engine's queue).
"""

import math
from contextlib import ExitStack

import numpy as np

import concourse.bass as bass
import concourse.bacc as bacc
import concourse.mybir as mybir
import concourse.tile as tile
from concourse.bass_utils import run_bass_kernel_spmd

F32 = mybir.dt.float32
BF16 = mybir.dt.bfloat16
FP8 = mybir.dt.float8e4
AX = mybir.AxisListType
ALU = mybir.AluOpType
ACTF = mybir.ActivationFunctionType
DR = mybir.MatmulPerfMode.DoubleRow

B, CI, CO, H, W, E, KK, RR = 16, 128, 128, 128, 128, 16, 3, 8
NCORES = 8
BL = B // NCORES  # 2 samples per core
EPS = 1e-5
HW = H * W
BNS = 1.0 / math.sqrt(1.0 + EPS)
HP, WP = H + 2, W + 2  # host-padded
HT = H + 6  # h dim of spsum/spmax with +-3 padding for the 7x7 conv

_CACHE = {}


def _build_module():
    nc = bacc.Bacc("TRN2", target_bir_lowering=False, debug=False)

    x8_d = nc.dram_tensor("x8", [BL, CI, HP, WP], FP8, kind="ExternalInput").ap()
    xt_d = nc.dram_tensor("xt", [BL, W, H, CI], BF16, kind="ExternalInput").ap()
    ew_d = nc.dram_tensor("ew", [128, KK * KK, 16, CI], FP8, kind="ExternalInput").ap()
    mc_d = nc.dram_tensor("mc", [128, 14, 128], BF16, kind="ExternalInput").ap()
    # packed constant blocks (single descriptor-efficient DMA each):
    # f32: pv(2) gs1 bb1 gs2n bb2n gsca1 bbca1 gsca2 bbca2 gssa bssa |
    #      rb3r(16, parts 0:2) rw1t(16) rw3t(16) caw1t(16) |
    #      rw2t(128, parts 0:16) caw2t(128, parts 0:16)
    cf_d = nc.dram_tensor("cf", [128, 332], F32, kind="ExternalInput").ap()
    # bf16: idc(129) | e16t(128, parts 0:16)
    cb_d = nc.dram_tensor("cb", [128, 257], BF16, kind="ExternalInput").ap()
    # fp8: s8(18) | bmask2(16)
    c8_d = nc.dram_tensor("c8", [128, 34], FP8, kind="ExternalInput").ap()

    out_d = nc.dram_tensor("out", [BL, W, H, CO], BF16, kind="ExternalOutput").ap()

    with tile.TileContext(nc) as tc, ExitStack() as ctx:
        _kernel_body(ctx, tc, x8_d, xt_d, ew_d, mc_d, cf_d, cb_d, c8_d, out_d)
    nc.compile()
    return nc


def _kernel_body(ctx, tc, x8_d, xt_d, ew_d, mc_d, cf_d, cb_d, c8_d, out_d):
    nc = tc.nc

    cpool = ctx.enter_context(tc.tile_pool(name="const", bufs=1))
    xpool = ctx.enter_context(tc.tile_pool(name="xp", bufs=2))
    opool = ctx.enter_context(tc.tile_pool(name="op", bufs=1))
    tpool = ctx.enter_context(tc.tile_pool(name="tp", bufs=2))
    wpool = ctx.enter_context(tc.tile_pool(name="wp", bufs=1))
    spool = ctx.enter_context(tc.tile_pool(name="sp", bufs=2))
    fpool = ctx.enter_context(tc.tile_pool(name="fp", bufs=4))

    pconv = ctx.enter_context(tc.tile_pool(name="pc", bufs=3, space="PSUM"))
    pbig = ctx.enter_context(tc.tile_pool(name="pb", bufs=2, space="PSUM"))
    psml = ctx.enter_context(tc.tile_pool(name="ps", bufs=1, space="PSUM"))

    # warm the exp ACT table set at t=0 (overlaps const DMA)
    wrm = spool.tile([1, 1], F32, tag="wrm", bufs=1)
    nc.vector.memset(wrm, 0.0)
    nc.scalar.activation(wrm, wrm, ACTF.Exp)

    # ---------- big loads all on the sync queue (scalar stays pure
    # compute: a DMA issue costs ~650ns of the issuing engine) ----------
    XCH = [(0, 33), (33, 65), (65, 97), (97, 130)]
    cf = cpool.tile([128, 332], F32, tag="cf")
    nc.sync.dma_start(cf, cf_d)
    x8s = []
    for b in range(BL):
        x8 = xpool.tile([128, HP, WP], FP8, tag="x8", name=f"x8_{b}")
        x8s.append(x8)
    for u, (r0, r1) in enumerate(XCH):
        nc.sync.dma_start(x8s[0][:, r0:r1, :], x8_d[0, :, r0:r1, :])
    # experts in tap-pair chunks so wgen starts early
    ecr = cpool.tile([128, KK * KK, 16, CI], FP8, tag="ecr")
    for kt in range(5):
        k1 = min(2 * kt + 2, 9)
        nc.sync.dma_start(ecr[:, 2 * kt:k1], ew_d[:, 2 * kt:k1])
    for u, (r0, r1) in enumerate(XCH):
        nc.sync.dma_start(x8s[1][:, r0:r1, :], x8_d[1, :, r0:r1, :])

    # ---------- packed constants (cf loaded first on sync above) ----------
    cb = cpool.tile([128, 257], BF16, tag="cb")
    nc.gpsimd.dma_start(cb, cb_d)
    c8 = cpool.tile([128, 34], FP8, tag="c8")
    nc.gpsimd.dma_start(c8, c8_d)
    mc = cpool.tile([128, 14, 128], BF16, tag="mc")
    nc.gpsimd.dma_start(mc, mc_d)

    pv = cf[:, 0:2]
    gs1, bb1 = cf[0:16, 2:3], cf[0:16, 3:4]
    gs2n, bb2n = cf[:, 4:5], cf[:, 5:6]
    gsca1, bbca1 = cf[0:16, 6:7], cf[0:16, 7:8]
    gsca2, bbca2 = cf[:, 8:9], cf[:, 9:10]
    gssa, bssa = cf[:, 10:11], cf[:, 11:12]
    rb3r = cf[0:BL, 12:28]
    rw1t = cf[:, 28:44]
    rw3t = cf[:, 44:60]
    caw1t = cf[:, 60:76]
    rw2t = cf[0:16, 76:204]
    caw2t = cf[0:16, 204:332]
    idc = cb[:, 0:129]
    e16t = cb[0:16, 129:257]
    s8 = c8[:, 0:18].rearrange("p (b k) -> p b k", b=BL)
    bmask = c8[:, 18:34]

    # ---------- routing (both samples batched; exp table set only) ----
    def routing():
        mm1 = psml.tile([16, BL], F32, tag="r", bufs=1)
        nc.tensor.matmul(mm1, rw1t, pv, start=True, stop=True)
        h1 = spool.tile([16, BL], F32, tag="h1")
        nc.scalar.activation(h1, mm1, ACTF.Relu, bias=bb1, scale=gs1)
        mm2 = psml.tile([128, BL], F32, tag="r", bufs=1)
        nc.tensor.matmul(mm2, rw2t, h1, start=True, stop=True)
        # sigmoid(v) = 1/(1+exp(-v)) to stay in the exp table set
        eg = spool.tile([128, BL], F32, tag="eg")
        nc.scalar.activation(eg, mm2, ACTF.Exp, bias=bb2n, scale=gs2n)
        gp = spool.tile([128, BL], F32, tag="gp")
        nc.scalar.activation(gp, eg, ACTF.Copy, bias=1.0)
        gg = spool.tile([128, BL], F32, tag="gg")
        nc.vector.reciprocal(gg, gp)
        mm3 = psml.tile([BL, E], F32, tag="r", bufs=1)
        nc.tensor.matmul(mm3, gg, rw3t, start=True, stop=True)
        lg = spool.tile([BL, E], F32, tag="lg")
        nc.vector.tensor_add(lg, mm3, rb3r)
        # logits are O(0.3) here, so plain exp is safe (no max-subtract)
        e16 = spool.tile([BL, E], F32, tag="e16")
        nc.scalar.activation(e16, lg, ACTF.Exp)
        s1 = spool.tile([BL, 1], F32, tag="s1")
        nc.vector.tensor_reduce(s1, e16, AX.X, ALU.add)
        rinv = spool.tile([BL, 1], F32, tag="rinv")
        nc.vector.reciprocal(rinv, s1)
        rwrow = spool.tile([BL, E], BF16, tag="rwrow")
        nc.vector.tensor_scalar_mul(rwrow, e16, rinv)
        # transpose [BL,16] -> [16,BL] on the PE, widen to 16 (j,b) cols
        rwtp = psml.tile([16, BL], BF16, tag="r", bufs=1)
        nc.tensor.matmul(rwtp, rwrow, idc[0:BL, 0:BL], is_transpose=True)
        rwt16 = spool.tile([16, 8, BL], BF16, tag="rwt16", bufs=1)
        nc.vector.tensor_copy(rwt16, rwtp.unsqueeze(1).broadcast_to([16, 8, BL]))
        rwbp = psml.tile([128, 16], F32, tag="r", bufs=1)
        nc.tensor.matmul(rwbp, e16t, rwt16, start=True, stop=True)
        rwblk = spool.tile([128, 16], FP8, tag="rwblk", bufs=1)
        nc.vector.tensor_tensor(rwblk, bmask, rwbp, ALU.mult)
        return rwblk

    # ---------- wgen (both samples): w[i, k, o] stored as w*32, fp8 ----
    def wgen(rwblk):
        wsbs = [wpool.tile([128, 9, CO], FP8, tag=f"wsb{b}", name=f"wsb{b}")
                for b in range(BL)]
        for kt in range(5):  # taps (2kt, 2kt+1); kt=4 -> tap 8 only
            ntap = 1 if kt == 4 else 2
            pwt = pbig.tile([128, 2, 512], F32, tag="big", name=f"pw{kt}")
            for j in range(ntap):
                k = 2 * kt + j
                for og in range(16):
                    nc.tensor.matmul(pwt[:, j, og * 16:og * 16 + 16],
                                     ecr[:, k, og, :], rwblk,
                                     start=True, stop=True)
                for b in range(BL):
                    src = bass.AP(pwt.tensor, pwt.offset + j * 512 + b,
                                  [list(pwt.ap[0]), [16, 16], [2, 8]])
                    nc.vector.tensor_scalar_mul(wsbs[b][:, k, :], src, 2.0)
        return wsbs

    # ---------- SE chain for sample b (analytic channel sums) ----------
    def se_chain(b, wsb):
        pcp = psml.tile([128, 1], F32, tag="r", bufs=1, name=f"pcp{b}")
        for k in range(9):
            nc.tensor.matmul(pcp, wsb[:, k, :], s8[:, b, k].unsqueeze(1),
                             start=(k == 0), stop=(k == 8))
        cpsb = spool.tile([128, 1], F32, tag="cpsb")
        nc.vector.tensor_copy(cpsb, pcp)
        se1 = psml.tile([16, 1], F32, tag="r", bufs=1, name=f"se1_{b}")
        nc.tensor.matmul(se1, caw1t, cpsb, start=True, stop=True)
        ch = spool.tile([16, 1], F32, tag="ch")
        nc.scalar.activation(ch, se1, ACTF.Relu, bias=bbca1, scale=gsca1)
        se2 = psml.tile([128, 1], F32, tag="r", bufs=1, name=f"se2_{b}")
        nc.tensor.matmul(se2, caw2t, ch, start=True, stop=True)
        cw = spool.tile([128, 1], F32, tag="cw")
        nc.scalar.activation(cw, se2, ACTF.Sigmoid, bias=bbca2, scale=gsca2)
        cws = spool.tile([128, 1], F32, tag="cws", name=f"cws{b}")
        nc.vector.tensor_scalar_mul(cws, cw, 1.0 / 32.0)
        return cws

    # conv tap pairs for DoubleRow (kw parity must match; rhs delta even)
    PAIRS = [(0, 3), (1, 4), (2, 5), (6, 8)]
    CH6 = [(6 * c, 6) for c in range(21)] + [(126, 2)]

    def conv_post(b, wsb, cws):
        """Conv rounds with the transpose/stats chunks interleaved, then
        the max tree + 7x7 spatial conv -> swT2."""
        x8 = x8s[b]
        osb = opool.tile([128, H, W], BF16, tag="osb", name=f"osb{b}")
        osbT = tpool.tile([128, H, CO], BF16, tag="osbT", name=f"osbT{b}")
        spsum = tpool.tile([128, HT], BF16, tag="spsum", name=f"spsum{b}")
        spmax = tpool.tile([128, HT], BF16, tag="spmax", name=f"spmax{b}")

        nc.vector.memset(spsum[:, 0:3], 0.0)
        nc.vector.memset(spsum[:, H + 3:H + 6], 0.0)
        nc.vector.memset(spmax[:, 0:3], 0.0)
        nc.vector.memset(spmax[:, H + 3:H + 6], 0.0)

        def emit_round(rnd):
            pcs = [pconv.tile([128, 512], F32, tag="c", name=f"pc{b}_{rnd}_{g}")
                   for g in range(2)]
            for t, (k0, k1) in enumerate(PAIRS):
                kh0, kw0 = k0 // 3, k0 % 3
                kh1, kw1 = k1 // 3, k1 % 3
                delta = (kh1 - kh0) * WP + (kw1 - kw0)
                lhs = bass.AP(wsb.tensor, wsb.offset + k0 * CO,
                              [list(wsb.ap[0]), [(k1 - k0) * CO, 2], [1, CO]])
                for g in range(2):
                    r0 = rnd * 8 + g * 4
                    eloff = (r0 + kh0) * WP + kw0
                    rhs = bass.AP(x8.tensor, x8.offset + eloff,
                                  [list(x8.ap[0]), [delta, 2], [WP, 4], [1, W]])
                    nc.tensor.matmul(pcs[g], lhs, rhs, start=(t == 0),
                                     stop=False, perf_mode=DR)
            # tap 7 (kh=2, kw=1) plain fp8 matmul (FWL path)
            for g in range(2):
                r0 = rnd * 8 + g * 4
                rhs7 = bass.AP(x8.tensor, x8.offset + (r0 + 2) * WP + 1,
                               [list(x8.ap[0]), [WP, 4], [1, W]])
                nc.tensor.matmul(pcs[g], wsb[:, 7, :], rhs7,
                                 start=False, stop=True)
            for g in range(2):
                r0 = rnd * 8 + g * 4
                nc.scalar.activation(
                    osb[:, r0:r0 + 4, :],
                    pcs[g].rearrange("p (a b) -> p a b", a=4),
                    ACTF.Copy, scale=cws)

        def emit_chunk(h0, nr):
            ptt = pbig.tile([128, 2, 512], F32, tag="big", name=f"ptt{b}_{h0}")
            for j in range(nr):
                nc.tensor.matmul(ptt[:, j // 3, (j % 3) * 129:(j % 3) * 129 + 129],
                                 osb[:, h0 + j, :], idc, start=True, stop=True)
            nb = (nr + 2) // 3
            nr3 = min(nr, 3)
            # o-columns -> osbT rows (contiguous), one ACT instr
            src = bass.AP(ptt.tensor, ptt.offset,
                          [list(ptt.ap[0]), [512, nb], [129, nr3], [1, CO]])
            dst = bass.AP(osbT.tensor, osbT.offset + h0 * CO,
                          [list(osbT.ap[0]), [3 * CO, nb], [CO, nr3], [1, CO]])
            nc.scalar.activation(dst, src, ACTF.Copy)
            # sum column -> spsum (tiny, on DVE)
            ssrc = bass.AP(ptt.tensor, ptt.offset + CO,
                           [list(ptt.ap[0]), [512, nb], [129, nr3]])
            sdst = bass.AP(spsum.tensor, spsum.offset + 3 + h0,
                           [list(spsum.ap[0]), [3, nb], [1, nr3]])
            nc.vector.tensor_copy(sdst, ssrc)
            # channel max: pairwise TT tree (TT max has a 2x uop;
            # reduce does not), fully reduced per chunk
            scr = tpool.tile([128, 6, 64], BF16, tag="scr", bufs=2,
                             name=f"scr{b}_{h0}")
            nc.vector.tensor_tensor(scr[:, 0:nr, :],
                                    osbT[:, h0:h0 + nr, 0:64],
                                    osbT[:, h0:h0 + nr, 64:128], ALU.max)
            wd = 32
            while wd >= 1:
                dst = (spmax[:, 3 + h0:3 + h0 + nr] if wd == 1
                       else scr[:, 0:nr, 0:wd])
                nc.vector.tensor_tensor(dst, scr[:, 0:nr, 0:wd],
                                        scr[:, 0:nr, wd:2 * wd], ALU.max)
                wd //= 2

        ci = 0
        for rnd in range(16):
            emit_round(rnd)
            while ci < len(CH6) and CH6[ci][0] + CH6[ci][1] <= 8 * rnd:
                emit_chunk(*CH6[ci])
                ci += 1
        while ci < len(CH6):
            emit_chunk(*CH6[ci])
            ci += 1

        # 7x7 spatial conv: 14 banded Toeplitz matmuls
        psw = pbig.tile([128, 2, 512], F32, tag="big", name=f"psw{b}")
        for t in range(14):
            c, dh = t // 7, t % 7
            src = spsum if c == 0 else spmax
            nc.tensor.matmul(psw[:, 0, 0:128], mc[:, t, :], src[:, dh:dh + 128],
                             start=(t == 0), stop=(t == 13))
        swT = spool.tile([128, 128], BF16, tag="swT", name=f"swT{b}")
        nc.scalar.activation(swT, psw[:, 0, 0:128], ACTF.Sigmoid,
                             bias=bssa, scale=gssa)
        swTf = spool.tile([128, 128], F32, tag="swTf", name=f"swTf{b}")
        nc.vector.tensor_copy(swTf, swT)
        # duplicated-pair copy so the final multiply can run in 2x mode:
        # swT2[w, h, 0:2] = swT[w, h]
        swT2 = spool.tile([128, 128, 2], BF16, tag="swT2", name=f"swT2{b}")
        nc.vector.tensor_copy(swT2, swT.unsqueeze(2).broadcast_to([128, 128, 2]))
        return osbT, swTf, swT2

    def final(b, osbT, swTf, swT2, nact=0):
        # the last `nact` 8-row tiles' multiplies run per-row on the (idle
        # in the tail) scalar engine via the per-partition f32 scale
        foas = []
        for g in range(16 - nact, 16):
            fo = fpool.tile([128, 8, CO], BF16, tag="foa", bufs=4,
                            name=f"foa{b}_{g}")
            for j in range(8):
                h = 8 * g + j
                nc.scalar.activation(fo[:, j, :], osbT[:, h, :], ACTF.Copy,
                                     scale=swTf[:, h:h + 1])
            foas.append(fo)
        nq = (16 - nact) * 8 // 32
        for q in range(nq):  # 32-row quarters
            h0 = 32 * q
            xtt = fpool.tile([128, 32, CI], BF16, tag="xtt", bufs=2,
                             name=f"xt{b}_{q}")
            nc.sync.dma_start(xtt, xt_d[b, :, h0:h0 + 32, :])
            fo = fpool.tile([128, 32, CO], BF16, tag="fo", bufs=2,
                            name=f"fo{b}_{q}")
            # operand2 reads each (w,h) sw value as duplicated bf16
            # pairs (innermost step 1) so 2x_1P packing applies
            op2 = bass.AP(swT2.tensor, swT2.offset + h0 * 2,
                          [list(swT2.ap[0]), [2, 32], [0, 64], [1, 2]])
            op1 = bass.AP(osbT.tensor, osbT.offset + h0 * CO,
                          [list(osbT.ap[0]), [CO, 32], [2, 64], [1, 2]])
            fov = bass.AP(fo.tensor, fo.offset,
                          [list(fo.ap[0]), [CO, 32], [2, 64], [1, 2]])
            nc.vector.tensor_tensor(fov, op1, op2, ALU.mult)
            nc.vector.tensor_tensor(fo, fo, xtt, ALU.add)
            nc.sync.dma_start(out_d[b, :, h0:h0 + 32, :], fo)
        for i, g in enumerate(range(16 - nact, 16)):
            h0 = 8 * g
            xtt = fpool.tile([128, 8, CI], BF16, tag="xta", bufs=2,
                             name=f"xta{b}_{g}")
            nc.sync.dma_start(xtt, xt_d[b, :, h0:h0 + 8, :])
            fo = foas[i]
            nc.vector.tensor_tensor(fo, fo, xtt, ALU.add)
            nc.gpsimd.dma_start(out_d[b, :, h0:h0 + 8, :], fo)

    # ---------- schedule ----------
    rwblk = routing()
    wsbs = wgen(rwblk)
    cws0 = se_chain(0, wsbs[0])
    cws1 = se_chain(1, wsbs[1])
    osbT0, swT0, swT20 = conv_post(0, wsbs[0], cws0)
    osbT1, swT1, swT21 = conv_post(1, wsbs[1], cws1)
    final(0, osbT0, swT0, swT20, nact=0)
    final(1, osbT1, swT1, swT21, nact=4)


def _host_prep(inp):
    import ml_dtypes
    x = np.asarray(inp["x"], np.float32)

    # padded fp8 conv input
    x8 = np.zeros((B, CI, HP, WP), dtype=ml_dtypes.float8_e4m3fn)
    x8[:, :, 1:H + 1, 1:W + 1] = x.astype(ml_dtypes.float8_e4m3fn)
    # transposed bf16 residual input [b, w, h, i]
    xt = np.ascontiguousarray(x.transpose(0, 3, 2, 1)).astype(ml_dtypes.bfloat16)

    # exact channel pool sums [i, b]
    xs = x.sum(axis=(2, 3))  # [B, I]
    # 3x3 window sums of the padded fp8 x (what the conv actually sees)
    x8f = np.zeros((B, CI, HP, WP), np.float32)
    x8f[:, :, 1:H + 1, 1:W + 1] = x8[:, :, 1:H + 1, 1:W + 1].astype(np.float32)
    r1 = x8f[:, :, 1, :].sum(-1)
    r128 = x8f[:, :, H, :].sum(-1)
    c1 = x8f[:, :, :, 1].sum(-1)
    c128 = x8f[:, :, :, W].sum(-1)
    tot = x8f.sum((2, 3))
    s9 = np.zeros((B, CI, 3, 3), np.float32)  # [b, i, kh, kw]
    for kh in range(3):
        a = tot.copy()
        if kh == 0:
            a -= r128
        if kh == 2:
            a -= r1
        for kw in range(3):
            v = a.copy()
            if kw == 0:
                d = c128.copy()
                if kh == 0:
                    d -= x8f[:, :, H, W]
                if kh == 2:
                    d -= x8f[:, :, 1, W]
                v -= d
            if kw == 2:
                d = c1.copy()
                if kh == 0:
                    d -= x8f[:, :, H, 1]
                if kh == 2:
                    d -= x8f[:, :, 1, 1]
                v -= d
            s9[:, :, kh, kw] = v
    s9 = s9 * 0.25

    experts = np.ascontiguousarray(inp["experts"], dtype=np.float32)
    # [E, O, I, K, K] -> [(o_sub 8, e 16)=128, k=9, og=16, i=128]
    ew = experts.reshape(E, 16, 8, CI, 9).transpose(2, 0, 4, 1, 3)
    ew = np.ascontiguousarray(ew).reshape(128, 9, 16, CI)

    idc = np.zeros((128, 129), dtype=np.float32)
    idc[np.arange(128), np.arange(128)] = 1.0
    idc[:, 128] = 1.0

    # banded Toeplitz matrices M[t=(c,dh)][w', w] = tap[c,dh,dw] at
    # w == w' + 3 - dw  (mean channel c=0 scaled by 1/CO)
    saw = np.asarray(inp["sa_w"], np.float32).reshape(2, 7, 7)
    mcm = np.zeros((14, 128, 128), dtype=np.float32)
    for t in range(14):
        c, dh = t // 7, t % 7
        for dw in range(7):
            val = float(saw[c, dh, dw]) * (1.0 / CO if c == 0 else 1.0)
            wp = np.arange(128)
            w = wp + 3 - dw
            m = (w >= 0) & (w < 128)
            mcm[t, wp[m], w[m]] += val
    mc = np.ascontiguousarray(mcm.transpose(1, 0, 2)).astype(ml_dtypes.bfloat16)

    e16t = np.zeros((16, 8, 16), dtype=np.float32)
    for e in range(16):
        e16t[e, :, e] = 1.0
    e16t = e16t.reshape(16, 128)

    # bmask2[p=(osub 8, e 16), col=(j*2+b)] = 1 iff osub == j
    bm = np.zeros((8, 16, 8, BL), dtype=np.float32)
    for j in range(8):
        bm[j, :, j, :] = 1.0
    bm = bm.reshape(128, 16)

    # ---- packed constant blocks ----
    cf = np.zeros((128, 332), np.float32)
    cf[0:16, 2] = np.asarray(inp["rbn1_g"], np.float32) * (BNS / HW)
    cf[0:16, 3] = np.asarray(inp["rbn1_b"], np.float32)
    cf[:, 4] = np.asarray(inp["rbn2_g"], np.float32) * (-BNS)
    cf[:, 5] = -np.asarray(inp["rbn2_b"], np.float32)
    cf[0:16, 6] = np.asarray(inp["ca_bn1_g"], np.float32) * (BNS / HW / 8.0)
    cf[0:16, 7] = np.asarray(inp["ca_bn1_b"], np.float32)
    cf[:, 8] = np.asarray(inp["ca_bn2_g"], np.float32) * BNS
    cf[:, 9] = np.asarray(inp["ca_bn2_b"], np.float32)
    cf[:, 10] = float(np.asarray(inp["sa_bn_g"], np.float32)[0]) * BNS
    cf[:, 11] = float(np.asarray(inp["sa_bn_b"], np.float32)[0])
    cf[0:BL, 12:28] = np.asarray(inp["rb3"], np.float32)[None, :]
    cf[:, 28:44] = np.asarray(inp["rw1"], np.float32).T
    cf[:, 44:60] = np.asarray(inp["rw3"], np.float32).T
    cf[:, 60:76] = np.asarray(inp["ca_w1"], np.float32).T
    cf[0:16, 76:204] = np.asarray(inp["rw2"], np.float32).T
    cf[0:16, 204:332] = np.asarray(inp["ca_w2"], np.float32).T

    cb = np.zeros((128, 257), np.float32)
    cb[:, 0:129] = idc
    cb[0:16, 129:257] = e16t
    cb = cb.astype(ml_dtypes.bfloat16)

    shared = {
        "ew": (ew * 16.0).astype(ml_dtypes.float8_e4m3fn),
        "mc": mc,
        "cb": cb,
    }
    in_maps = []
    for c in range(NCORES):
        m = dict(shared)
        sl = slice(BL * c, BL * (c + 1))
        m["x8"] = np.ascontiguousarray(x8[sl])
        m["xt"] = np.ascontiguousarray(xt[sl])
        cfc = cf.copy()
        cfc[:, 0:2] = xs[sl].T  # pv [i, b]
        m["cf"] = cfc
        c8 = np.zeros((128, 34), np.float32)
        # s8 layout [i, b, 9] with col k = kh*3+kw
        c8[:, 0:18] = s9[sl].transpose(1, 0, 2, 3).reshape(CI, BL * 9)
        c8[:, 18:34] = bm
        m["c8"] = c8.astype(ml_dtypes.float8_e4m3fn)
        in_maps.append(m)
    return in_maps


def _assemble(results):
    out = np.concatenate([r["out"] for r in results], axis=0)  # [B, W, H, O]
    return np.ascontiguousarray(out.transpose(0, 3, 2, 1)).astype(np.float32)


def get_module():
    if "nc" not in _CACHE:
        _CACHE["nc"] = _build_module()
    return _CACHE["nc"]


def kernel(**inputs):
    nc = get_module()
    in_maps = _host_prep(inputs)
    res = run_bass_kernel_spmd(nc, in_maps, core_ids=list(range(NCORES)))
    return _assemble(res.results)
